# revision 6
# baseline (speedup 1.0000x reference)
"""Trainium2 Bass kernel for CenterWoParamMultiCosineNearLoss.

loss = mean_b [ S_b - m_b + (2*m_b^2 - Q_b) / S_b ]   where, per sample b,
  d_k = 1 - <x_b, c_{label_b, k}>  (k = 0..15 sub-centers of own class)
  S = sum_k d_k, Q = sum_k d_k^2, m = min_k d_k
(algebraically identical to the reference's term1+term2; verified exactly).

Sharding: samples are sorted by label on the host (the loss is a mean over
samples, hence permutation-invariant) and split into 8 contiguous shards of
1024 — i.e. data-parallel with class-clustered assignment. Each core's shard
then spans only ~13 consecutive classes, so the device matmul per core is
x_shard^T [1024d x 1024b] against a small window of transposed centers
[1024d x 16*W cols] instead of all 90*16=1440 columns. Per-row selection of
the 16 own-class columns is done on device with an iota==label one-hot mask
and a strided reduction. Each core emits its partial row-loss sum; the host
all-reduces the 8 scalars into the mean.
"""

import os
import sys

import numpy as np

for _p in ("/opt/trn_rl_repo", "/root/.axon_site/_ro/trn_rl_repo"):
    if os.path.isdir(_p) and _p not in sys.path:
        sys.path.append(_p)

import concourse.bass as bass  # noqa: E402
import concourse.tile as tile  # noqa: E402
from concourse import bacc  # noqa: E402
from concourse import mybir  # noqa: E402
from concourse.bass_utils import run_bass_kernel_spmd  # noqa: E402

P = 128          # SBUF partitions
B = 8192         # batch
D = 1024         # feature dim
C = 90           # classes
K = 16           # sub-centers per class
NCORES = 8
SHARD = B // NCORES          # 1024 samples per core
NB = SHARD // P              # 8 row-blocks per core
KT = D // P                  # 8 contraction tiles

_F32 = mybir.dt.float32
_F32R = mybir.dt.float32r
_I32 = mybir.dt.int32


def _build_program(wc: int):
    """One SPMD program for all 8 cores. wc = window width in columns (16*W)."""
    w = wc // K
    nc = bacc.Bacc(None, target_bir_lowering=False)
    xT = nc.declare_dram_parameter("xT", [D, SHARD], _F32R, isOutput=False)
    cw = nc.declare_dram_parameter("cw", [D, wc], _F32R, isOutput=False)
    lab = nc.declare_dram_parameter("lab", [P, NB], _F32, isOutput=False)
    out = nc.declare_dram_parameter("out", [1, 1], _F32, isOutput=True)

    with tile.TileContext(nc) as tc:
        with (
            tc.tile_pool(name="const", bufs=1) as const,
            tc.tile_pool(name="cwp", bufs=1) as cwp,
            tc.tile_pool(name="xp", bufs=3) as xp,
            tc.tile_pool(name="work", bufs=3) as work,
            tc.tile_pool(name="stats", bufs=1) as stats,
            tc.tile_pool(name="pp", bufs=4, space="PSUM") as pp,
            tc.tile_pool(name="ppf", bufs=1, space="PSUM") as ppf,
        ):
            # constants
            colc = const.tile([P, wc], _F32)
            nc.gpsimd.iota(
                colc[:, :], pattern=[[1, w], [0, K]], channel_multiplier=0,
                allow_small_or_imprecise_dtypes=True,
            )
            ones = const.tile([P, 1], _F32)
            nc.vector.memset(ones[:, :], 1.0)
            # small DMA via SWDGE (single queue/sem) — an HWDGE transfer fans
            # out across queues and overflows the consumer's wait slots
            labt = const.tile([P, NB], _F32)
            nc.gpsimd.dma_start(out=labt[:, :], in_=lab[:, :])

            # all 8 k-tiles of the centers window: [d_local, ktile, col]
            cwt = cwp.tile([P, KT, wc], _F32R)
            nc.sync.dma_start(
                out=cwt[:, :, :], in_=cw[:, :].rearrange("(k p) w -> p k w", p=P)
            )

            ssum = stats.tile([P, NB], _F32)   # per-row sum of selected cos
            qsum = stats.tile([P, NB], _F32)   # per-row sum of selected cos^2
            mx = stats.tile([P, NB], _F32)     # per-row max of selected cos

            for i in range(NB):
                xbt = xp.tile([P, KT, P], _F32R)
                nc.sync.dma_start(
                    out=xbt[:, :, :],
                    in_=xT[:, i * P : (i + 1) * P].rearrange("(k p) b -> p k b", p=P),
                )
                ps = pp.tile([P, wc], _F32)
                for k in range(KT):
                    nc.tensor.matmul(
                        ps[:, :],
                        lhsT=xbt[:, k, :],
                        rhs=cwt[:, k, :],
                        start=(k == 0),
                        stop=(k == KT - 1),
                    )
                # one-hot over window classes for each row, expanded to 16 cols
                mask = work.tile([P, wc], _F32)
                nc.gpsimd.tensor_scalar(
                    out=mask[:, :],
                    in0=colc[:, :],
                    scalar1=labt[:, i : i + 1],
                    scalar2=None,
                    op0=mybir.AluOpType.is_equal,
                )
                sm = work.tile([P, wc], _F32)
                nc.vector.tensor_tensor(
                    out=sm[:, :], in0=ps[:, :], in1=mask[:, :], op=mybir.AluOpType.mult
                )
                # collapse the class axis (stride K) -> the 16 selected cos values
                dsel = work.tile([P, K], _F32)
                nc.vector.tensor_reduce(
                    out=dsel[:, :],
                    in_=sm[:, :].rearrange("p (c k) -> p k c", k=K),
                    axis=mybir.AxisListType.X,
                    op=mybir.AluOpType.add,
                )
                sq = work.tile([P, K], _F32)
                nc.vector.tensor_tensor(
                    out=sq[:, :], in0=dsel[:, :], in1=dsel[:, :], op=mybir.AluOpType.mult
                )
                nc.vector.tensor_reduce(
                    out=ssum[:, i : i + 1], in_=dsel[:, :],
                    axis=mybir.AxisListType.X, op=mybir.AluOpType.add,
                )
                nc.vector.tensor_reduce(
                    out=qsum[:, i : i + 1], in_=sq[:, :],
                    axis=mybir.AxisListType.X, op=mybir.AluOpType.add,
                )
                nc.vector.tensor_reduce(
                    out=mx[:, i : i + 1], in_=dsel[:, :],
                    axis=mybir.AxisListType.X, op=mybir.AluOpType.max,
                )

            # epilogue on [P, NB]: d = 1 - s  =>
            #   S = K - ssum; Q = K - 2*ssum + qsum; m = 1 - mx
            #   rowloss = S - m + (2*m^2 - Q) / S
            sd = stats.tile([P, NB], _F32)
            nc.vector.tensor_scalar(
                out=sd[:, :], in0=ssum[:, :], scalar1=-1.0, scalar2=float(K),
                op0=mybir.AluOpType.mult, op1=mybir.AluOpType.add,
            )
            t = stats.tile([P, NB], _F32)
            nc.vector.tensor_scalar(
                out=t[:, :], in0=ssum[:, :], scalar1=-2.0, scalar2=float(K),
                op0=mybir.AluOpType.mult, op1=mybir.AluOpType.add,
            )
            qd = stats.tile([P, NB], _F32)
            nc.vector.tensor_tensor(
                out=qd[:, :], in0=t[:, :], in1=qsum[:, :], op=mybir.AluOpType.add
            )
            md = stats.tile([P, NB], _F32)
            nc.vector.tensor_scalar(
                out=md[:, :], in0=mx[:, :], scalar1=-1.0, scalar2=1.0,
                op0=mybir.AluOpType.mult, op1=mybir.AluOpType.add,
            )
            m2 = stats.tile([P, NB], _F32)
            nc.vector.tensor_tensor(
                out=m2[:, :], in0=md[:, :], in1=md[:, :], op=mybir.AluOpType.mult
            )
            num = stats.tile([P, NB], _F32)
            nc.vector.tensor_scalar(
                out=num[:, :], in0=m2[:, :], scalar1=2.0, scalar2=None,
                op0=mybir.AluOpType.mult,
            )
            num2 = stats.tile([P, NB], _F32)
            nc.vector.tensor_tensor(
                out=num2[:, :], in0=num[:, :], in1=qd[:, :], op=mybir.AluOpType.subtract
            )
            rs = stats.tile([P, NB], _F32)
            nc.vector.reciprocal(rs[:, :], sd[:, :])
            frac = stats.tile([P, NB], _F32)
            nc.vector.tensor_tensor(
                out=frac[:, :], in0=num2[:, :], in1=rs[:, :], op=mybir.AluOpType.mult
            )
            base = stats.tile([P, NB], _F32)
            nc.vector.tensor_tensor(
                out=base[:, :], in0=sd[:, :], in1=md[:, :], op=mybir.AluOpType.subtract
            )
            rloss = stats.tile([P, NB], _F32)
            nc.vector.tensor_tensor(
                out=rloss[:, :], in0=base[:, :], in1=frac[:, :], op=mybir.AluOpType.add
            )
            rowsum = stats.tile([P, 1], _F32)
            nc.vector.tensor_reduce(
                out=rowsum[:, :], in_=rloss[:, :],
                axis=mybir.AxisListType.X, op=mybir.AluOpType.add,
            )
            # cross-partition sum via ones-matmul: [1,1] = rowsum^T @ ones
            psc = ppf.tile([1, 1], _F32)
            nc.tensor.matmul(
                psc[:, :], lhsT=rowsum[:, :], rhs=ones[:, :], start=True, stop=True
            )
            outsb = stats.tile([1, 1], _F32)
            nc.vector.tensor_copy(out=outsb[:, :], in_=psc[:, :])
            nc.sync.dma_start(out=out[:, :], in_=outsb[:, :])

    nc.finalize()  # Bacc: runs wait-splitting + register allocation passes
    return nc


def _prep_inputs(x, labels, centers):
    """Host-side sharding/layout prep. Returns (in_maps, wc)."""
    labels = np.asarray(labels).astype(np.int64)
    x = np.ascontiguousarray(np.asarray(x, dtype=np.float32))
    centers = np.asarray(centers, dtype=np.float32)

    perm = np.argsort(labels, kind="stable")
    ls = labels[perm]

    # per-core class windows
    starts, spans = [], []
    for i in range(NCORES):
        seg = ls[i * SHARD : (i + 1) * SHARD]
        lo, hi = int(seg[0]), int(seg[-1])
        starts.append(lo)
        spans.append(hi - lo + 1)
    w = max(max(spans), 16)  # >=16 classes so matmul N >= 256 (fp32r full rate)
    assert w * K <= 512, f"class span {w} too large for single PSUM bank"
    wc = w * K
    starts = [min(s, C - w) for s in starts]

    centersT = np.ascontiguousarray(centers.reshape(C * K, D).T)  # [D, C*K]

    in_maps = []
    for i in range(NCORES):
        rows = perm[i * SHARD : (i + 1) * SHARD]
        xsT = np.ascontiguousarray(x[rows].T)                     # [D, SHARD]
        cwin = np.ascontiguousarray(
            centersT[:, K * starts[i] : K * (starts[i] + w)]
        )                                                          # [D, wc]
        lab_local = (ls[i * SHARD : (i + 1) * SHARD] - starts[i]).astype(np.float32)
        lab_dev = np.ascontiguousarray(lab_local.reshape(NB, P).T)  # [P, NB]
        in_maps.append({"xT": xsT, "cw": cwin, "lab": lab_dev})
    return in_maps, wc


def kernel(x, labels, centers):
    in_maps, wc = _prep_inputs(x, labels, centers)
    nc = _build_program(wc)
    res = run_bass_kernel_spmd(nc, in_maps, core_ids=list(range(NCORES)))
    total = sum(float(r["out"][0, 0]) for r in res.results)
    return np.float32(total / B)


# revision 7
# speedup vs baseline: 1.3394x; 1.3394x over previous
"""Trainium2 Bass kernel for CenterWoParamMultiCosineNearLoss.

loss = mean_b [ S_b - m_b + (2*m_b^2 - Q_b) / S_b ]   where, per sample b,
  d_k = 1 - <x_b, c_{label_b, k}>  (k = 0..15 sub-centers of own class)
  S = sum_k d_k, Q = sum_k d_k^2, m = min_k d_k
(algebraically identical to the reference's term1+term2; verified exactly).

Sharding: samples are sorted by label on the host (the loss is a mean over
samples, hence permutation-invariant) and split into 8 contiguous shards of
1024 — i.e. data-parallel with class-clustered assignment. Each core's shard
then spans only ~13 consecutive classes, so the device matmul per core is
x_shard^T [1024d x 1024b] against a small window of transposed centers
[1024d x 16*W cols] instead of all 90*16=1440 columns. Per-row selection of
the 16 own-class columns is done on device with an iota==label one-hot mask
and a reduction over the class axis (window columns are laid out k-major so
the reduce is contiguous). Per-row sum/sumsq go through the scalar engine's
accumulate path; each core emits its partial row-loss sum and the host
all-reduces the 8 scalars into the mean.
"""

import os
import sys

import numpy as np

for _p in ("/opt/trn_rl_repo", "/root/.axon_site/_ro/trn_rl_repo"):
    if os.path.isdir(_p) and _p not in sys.path:
        sys.path.append(_p)

import concourse.bass as bass  # noqa: E402
import concourse.tile as tile  # noqa: E402
from concourse import bacc  # noqa: E402
from concourse import mybir  # noqa: E402
from concourse.bass_utils import run_bass_kernel_spmd  # noqa: E402

P = 128          # SBUF partitions
B = 8192         # batch
D = 1024         # feature dim
C = 90           # classes
K = 16           # sub-centers per class
NCORES = 8
SHARD = B // NCORES          # 1024 samples per core
NB = SHARD // P              # 8 row-blocks per core
KT = D // P                  # 8 contraction tiles

_F32 = mybir.dt.float32
_F32R = mybir.dt.float32r

_ADD = mybir.AluOpType.add
_MULT = mybir.AluOpType.mult
_SUB = mybir.AluOpType.subtract
_MAX = mybir.AluOpType.max
_EQ = mybir.AluOpType.is_equal
_AX = mybir.AxisListType.X


def _build_program(wc: int):
    """One SPMD program for all 8 cores. wc = window width in columns (K*w)."""
    w = wc // K
    nc = bacc.Bacc(None, target_bir_lowering=False)
    xT = nc.declare_dram_parameter("xT", [D, SHARD], _F32R, isOutput=False)
    cw = nc.declare_dram_parameter("cw", [D, wc], _F32R, isOutput=False)
    lab = nc.declare_dram_parameter("lab", [P, NB], _F32, isOutput=False)
    out = nc.declare_dram_parameter("out", [1, 1], _F32, isOutput=True)

    with tile.TileContext(nc) as tc:
        with (
            tc.tile_pool(name="const", bufs=1) as const,
            tc.tile_pool(name="cwp", bufs=1) as cwp,
            tc.tile_pool(name="xp", bufs=3) as xp,
            tc.tile_pool(name="maskp", bufs=NB) as maskp,
            tc.tile_pool(name="work", bufs=4) as work,
            tc.tile_pool(name="stats", bufs=1) as stats,
            tc.tile_pool(name="pp", bufs=6, space="PSUM") as pp,
            tc.tile_pool(name="ppf", bufs=1, space="PSUM") as ppf,
        ):
            # constants.  window columns are k-major: col j = k*w + c, so the
            # class id at column j is (j mod w)
            colc = const.tile([P, wc], _F32)
            nc.gpsimd.iota(
                colc[:, :], pattern=[[0, K], [1, w]], channel_multiplier=0,
                allow_small_or_imprecise_dtypes=True,
            )
            ones = const.tile([P, 1], _F32)
            nc.vector.memset(ones[:, :], 1.0)
            # small DMA via SWDGE (single queue/sem) — an HWDGE transfer fans
            # out across queues and overflows the consumer's wait slots
            labt = const.tile([P, NB], _F32)
            nc.gpsimd.dma_start(out=labt[:, :], in_=lab[:, :])

            # all 8 k-tiles of the centers window: [d_local, ktile, col]
            cwt = cwp.tile([P, KT, wc], _F32R)
            nc.sync.dma_start(
                out=cwt[:, :, :], in_=cw[:, :].rearrange("(k p) w -> p k w", p=P)
            )

            # one-hot masks for every block up-front (only depend on labt)
            masks = []
            for i in range(NB):
                mask = maskp.tile([P, wc], _F32)
                nc.vector.tensor_scalar(
                    out=mask[:, :], in0=colc[:, :], scalar1=labt[:, i : i + 1],
                    scalar2=None, op0=_EQ,
                )
                masks.append(mask)

            ssum = stats.tile([P, NB], _F32)   # per-row sum of selected cos
            qsum = stats.tile([P, NB], _F32)   # per-row sum of selected cos^2
            mx = stats.tile([P, NB], _F32)     # per-row max of selected cos

            for j in range(NB // 2):
                # two row-blocks per DMA: 1KB contiguous runs instead of 512B
                xbt = xp.tile([P, KT, 2 * P], _F32R)
                nc.sync.dma_start(
                    out=xbt[:, :, :],
                    in_=xT[:, j * 2 * P : (j + 1) * 2 * P].rearrange(
                        "(k p) b -> p k b", p=P
                    ),
                )
                for h in range(2):
                    i = 2 * j + h
                    ps = pp.tile([P, wc], _F32)
                    for k in range(KT):
                        nc.tensor.matmul(
                            ps[:, :],
                            lhsT=xbt[:, k, h * P : (h + 1) * P],
                            rhs=cwt[:, k, :],
                            start=(k == 0),
                            stop=(k == KT - 1),
                        )
                    sm = work.tile([P, wc], _F32)
                    nc.vector.tensor_tensor(
                        out=sm[:, :], in0=ps[:, :], in1=masks[i][:, :], op=_MULT
                    )
                    # collapse the class axis (contiguous, k-major layout)
                    dsel = work.tile([P, K], _F32)
                    nc.vector.tensor_reduce(
                        out=dsel[:, :],
                        in_=sm[:, :].rearrange("p (k c) -> p k c", c=w),
                        axis=_AX, op=_ADD,
                    )
                    # sum and sum-of-squares on the (idle) scalar engine via
                    # its accumulate path; max stays on vector
                    sq = work.tile([P, K], _F32)
                    nc.scalar.activation(
                        out=sq[:, :], in_=dsel[:, :],
                        func=mybir.ActivationFunctionType.Square,
                        accum_out=qsum[:, i : i + 1],
                    )
                    cp = work.tile([P, K], _F32)
                    nc.scalar.activation(
                        out=cp[:, :], in_=dsel[:, :],
                        func=mybir.ActivationFunctionType.Copy,
                        accum_out=ssum[:, i : i + 1],
                    )
                    nc.vector.tensor_reduce(
                        out=mx[:, i : i + 1], in_=dsel[:, :], axis=_AX, op=_MAX,
                    )

            # epilogue on [P, NB]: d = 1 - s  =>
            #   S = K - ssum; Q = K - 2*ssum + qsum; m = 1 - mx
            #   rowloss = S - m + (2*m^2 - Q) / S
            sd = stats.tile([P, NB], _F32)
            nc.vector.tensor_scalar(
                out=sd[:, :], in0=ssum[:, :], scalar1=-1.0, scalar2=float(K),
                op0=_MULT, op1=_ADD,
            )
            t = stats.tile([P, NB], _F32)
            nc.vector.tensor_scalar(
                out=t[:, :], in0=ssum[:, :], scalar1=-2.0, scalar2=float(K),
                op0=_MULT, op1=_ADD,
            )
            qd = stats.tile([P, NB], _F32)
            nc.vector.tensor_tensor(out=qd[:, :], in0=t[:, :], in1=qsum[:, :], op=_ADD)
            md = stats.tile([P, NB], _F32)
            nc.vector.tensor_scalar(
                out=md[:, :], in0=mx[:, :], scalar1=-1.0, scalar2=1.0,
                op0=_MULT, op1=_ADD,
            )
            m2 = stats.tile([P, NB], _F32)
            nc.vector.tensor_tensor(out=m2[:, :], in0=md[:, :], in1=md[:, :], op=_MULT)
            num = stats.tile([P, NB], _F32)
            nc.vector.tensor_scalar(
                out=num[:, :], in0=m2[:, :], scalar1=2.0, scalar2=None, op0=_MULT,
            )
            num2 = stats.tile([P, NB], _F32)
            nc.vector.tensor_tensor(out=num2[:, :], in0=num[:, :], in1=qd[:, :], op=_SUB)
            rs = stats.tile([P, NB], _F32)
            nc.vector.reciprocal(rs[:, :], sd[:, :])
            frac = stats.tile([P, NB], _F32)
            nc.vector.tensor_tensor(out=frac[:, :], in0=num2[:, :], in1=rs[:, :], op=_MULT)
            base = stats.tile([P, NB], _F32)
            nc.vector.tensor_tensor(out=base[:, :], in0=sd[:, :], in1=md[:, :], op=_SUB)
            rloss = stats.tile([P, NB], _F32)
            nc.vector.tensor_tensor(out=rloss[:, :], in0=base[:, :], in1=frac[:, :], op=_ADD)
            rowsum = stats.tile([P, 1], _F32)
            nc.vector.tensor_reduce(out=rowsum[:, :], in_=rloss[:, :], axis=_AX, op=_ADD)
            # cross-partition sum via ones-matmul: [1,1] = rowsum^T @ ones
            psc = ppf.tile([1, 1], _F32)
            nc.tensor.matmul(
                psc[:, :], lhsT=rowsum[:, :], rhs=ones[:, :], start=True, stop=True
            )
            outsb = stats.tile([1, 1], _F32)
            nc.vector.tensor_copy(out=outsb[:, :], in_=psc[:, :])
            nc.sync.dma_start(out=out[:, :], in_=outsb[:, :])

    nc.finalize()  # Bacc: runs wait-splitting + register allocation passes
    return nc


def _prep_inputs(x, labels, centers):
    """Host-side sharding/layout prep. Returns (in_maps, wc)."""
    labels = np.asarray(labels).astype(np.int64)
    x = np.ascontiguousarray(np.asarray(x, dtype=np.float32))
    centers = np.asarray(centers, dtype=np.float32)

    perm = np.argsort(labels, kind="stable")
    ls = labels[perm]

    # per-core class windows
    starts, spans = [], []
    for i in range(NCORES):
        seg = ls[i * SHARD : (i + 1) * SHARD]
        lo, hi = int(seg[0]), int(seg[-1])
        starts.append(lo)
        spans.append(hi - lo + 1)
    w = max(max(spans), 16)  # >=16 classes so matmul N >= 256 (fp32r full rate)
    assert w * K <= 512, f"class span {w} too large for single PSUM bank"
    wc = w * K
    starts = [min(s, C - w) for s in starts]

    centersT = np.ascontiguousarray(centers.reshape(C * K, D).T)  # [D, C*K]

    in_maps = []
    for i in range(NCORES):
        rows = perm[i * SHARD : (i + 1) * SHARD]
        xsT = np.ascontiguousarray(x[rows].T)                     # [D, SHARD]
        # window, k-major columns: col j = k*w + c  ->  centersT col 16*(start+c)+k
        win = centersT[:, K * starts[i] : K * (starts[i] + w)]     # [D, w*K] c-major
        cwin = np.ascontiguousarray(
            win.reshape(D, w, K).transpose(0, 2, 1).reshape(D, wc)
        )
        lab_local = (ls[i * SHARD : (i + 1) * SHARD] - starts[i]).astype(np.float32)
        lab_dev = np.ascontiguousarray(lab_local.reshape(NB, P).T)  # [P, NB]
        in_maps.append({"xT": xsT, "cw": cwin, "lab": lab_dev})
    return in_maps, wc


def kernel(x, labels, centers):
    in_maps, wc = _prep_inputs(x, labels, centers)
    nc = _build_program(wc)
    res = run_bass_kernel_spmd(nc, in_maps, core_ids=list(range(NCORES)))
    total = sum(float(r["out"][0, 0]) for r in res.results)
    return np.float32(total / B)


# revision 8
# speedup vs baseline: 1.4551x; 1.0864x over previous
"""Trainium2 Bass kernel for CenterWoParamMultiCosineNearLoss.

loss = mean_b [ S_b - m_b + (2*m_b^2 - Q_b) / S_b ]   where, per sample b,
  d_k = 1 - <x_b, c_{label_b, k}>  (k = 0..15 sub-centers of own class)
  S = sum_k d_k, Q = sum_k d_k^2, m = min_k d_k
(algebraically identical to the reference's term1+term2; verified exactly).

Sharding: samples are sorted by label on the host (the loss is a mean over
samples, hence permutation-invariant) and split into 8 contiguous shards of
1024 — i.e. data-parallel with class-clustered assignment. Each core's shard
then spans only ~13 consecutive classes, so the device matmul per core is
x_shard^T [1024d x 1024b] against a small window of transposed centers
[1024d x 16*W cols] instead of all 90*16=1440 columns. Per-row selection of
the 16 own-class columns is done on device with an iota==label one-hot mask
and a reduction over the class axis (window columns are laid out k-major so
the reduce is contiguous). Per-row sum/sumsq go through the scalar engine's
accumulate path; each core emits its partial row-loss sum and the host
all-reduces the 8 scalars into the mean.
"""

import os
import sys

import numpy as np

for _p in ("/opt/trn_rl_repo", "/root/.axon_site/_ro/trn_rl_repo"):
    if os.path.isdir(_p) and _p not in sys.path:
        sys.path.append(_p)

import concourse.bass as bass  # noqa: E402
import concourse.tile as tile  # noqa: E402
from concourse import bacc  # noqa: E402
from concourse import mybir  # noqa: E402
from concourse.bass_utils import run_bass_kernel_spmd  # noqa: E402

P = 128          # SBUF partitions
B = 8192         # batch
D = 1024         # feature dim
C = 90           # classes
K = 16           # sub-centers per class
NCORES = 8
SHARD = B // NCORES          # 1024 samples per core
NB = SHARD // P              # 8 row-blocks per core
KT = D // P                  # 8 contraction tiles

_F32 = mybir.dt.float32
_F32R = mybir.dt.float32r

_ADD = mybir.AluOpType.add
_MULT = mybir.AluOpType.mult
_SUB = mybir.AluOpType.subtract
_MAX = mybir.AluOpType.max
_EQ = mybir.AluOpType.is_equal
_AX = mybir.AxisListType.X


def _build_program(wc: int):
    """One SPMD program for all 8 cores. wc = window width in columns (K*w)."""
    w = wc // K
    nc = bacc.Bacc(None, target_bir_lowering=False)
    xT = nc.declare_dram_parameter("xT", [D, SHARD], _F32R, isOutput=False)
    cw = nc.declare_dram_parameter("cw", [D, wc], _F32R, isOutput=False)
    lab = nc.declare_dram_parameter("lab", [P, NB], _F32, isOutput=False)
    out = nc.declare_dram_parameter("out", [1, 1], _F32, isOutput=True)

    with tile.TileContext(nc) as tc:
        with (
            tc.tile_pool(name="const", bufs=1) as const,
            tc.tile_pool(name="cwp", bufs=1) as cwp,
            tc.tile_pool(name="xp", bufs=4) as xp,
            tc.tile_pool(name="maskp", bufs=NB) as maskp,
            tc.tile_pool(name="work", bufs=4) as work,
            tc.tile_pool(name="stats", bufs=1) as stats,
            tc.tile_pool(name="pp", bufs=6, space="PSUM") as pp,
            tc.tile_pool(name="ppf", bufs=1, space="PSUM") as ppf,
        ):
            # constants.  window columns are k-major: col j = k*w + c, so the
            # class id at column j is (j mod w)
            colc = const.tile([P, wc], _F32)
            nc.gpsimd.iota(
                colc[:, :], pattern=[[0, K], [1, w]], channel_multiplier=0,
                allow_small_or_imprecise_dtypes=True,
            )
            ones = const.tile([P, 1], _F32)
            nc.vector.memset(ones[:, :], 1.0)
            # small DMA via SWDGE (single queue/sem) — an HWDGE transfer fans
            # out across queues and overflows the consumer's wait slots
            labt = const.tile([P, NB], _F32)
            nc.gpsimd.dma_start(out=labt[:, :], in_=lab[:, :])

            # all 8 k-tiles of the centers window: [d_local, ktile, col]
            cwt = cwp.tile([P, KT, wc], _F32R)
            nc.sync.dma_start(
                out=cwt[:, :, :], in_=cw[:, :].rearrange("(k p) w -> p k w", p=P)
            )

            # one-hot masks for every block up-front (only depend on labt)
            masks = []
            for i in range(NB):
                mask = maskp.tile([P, wc], _F32)
                nc.vector.tensor_scalar(
                    out=mask[:, :], in0=colc[:, :], scalar1=labt[:, i : i + 1],
                    scalar2=None, op0=_EQ,
                )
                masks.append(mask)

            ssum = stats.tile([P, NB], _F32)   # per-row sum of selected cos
            qsum = stats.tile([P, NB], _F32)   # per-row sum of selected cos^2
            mx = stats.tile([P, NB], _F32)     # per-row max of selected cos

            for j in range(NB // 2):
                # two row-blocks per DMA: 1KB contiguous runs instead of 512B
                xbt = xp.tile([P, KT, 2 * P], _F32R)
                nc.sync.dma_start(
                    out=xbt[:, :, :],
                    in_=xT[:, j * 2 * P : (j + 1) * 2 * P].rearrange(
                        "(k p) b -> p k b", p=P
                    ),
                )
                for h in range(2):
                    i = 2 * j + h
                    ps = pp.tile([P, wc], _F32)
                    for k in range(KT):
                        nc.tensor.matmul(
                            ps[:, :],
                            lhsT=xbt[:, k, h * P : (h + 1) * P],
                            rhs=cwt[:, k, :],
                            start=(k == 0),
                            stop=(k == KT - 1),
                        )
                    sm = work.tile([P, wc], _F32)
                    nc.vector.tensor_tensor(
                        out=sm[:, :], in0=ps[:, :], in1=masks[i][:, :], op=_MULT
                    )
                    # collapse the class axis (contiguous, k-major layout)
                    dsel = work.tile([P, K], _F32)
                    nc.vector.tensor_reduce(
                        out=dsel[:, :],
                        in_=sm[:, :].rearrange("p (k c) -> p k c", c=w),
                        axis=_AX, op=_ADD,
                    )
                    # sum and sum-of-squares on the (idle) scalar engine via
                    # its accumulate path; max stays on vector
                    sq = work.tile([P, K], _F32)
                    nc.scalar.activation(
                        out=sq[:, :], in_=dsel[:, :],
                        func=mybir.ActivationFunctionType.Square,
                        accum_out=qsum[:, i : i + 1],
                    )
                    cp = work.tile([P, K], _F32)
                    nc.scalar.activation(
                        out=cp[:, :], in_=dsel[:, :],
                        func=mybir.ActivationFunctionType.Copy,
                        accum_out=ssum[:, i : i + 1],
                    )
                    nc.vector.tensor_reduce(
                        out=mx[:, i : i + 1], in_=dsel[:, :], axis=_AX, op=_MAX,
                    )

            # epilogue on [P, NB]: d = 1 - s  =>
            #   S = K - ssum; Q = K - 2*ssum + qsum; m = 1 - mx
            #   rowloss = S - m + (2*m^2 - Q) / S
            sd = stats.tile([P, NB], _F32)
            nc.vector.tensor_scalar(
                out=sd[:, :], in0=ssum[:, :], scalar1=-1.0, scalar2=float(K),
                op0=_MULT, op1=_ADD,
            )
            t = stats.tile([P, NB], _F32)
            nc.vector.tensor_scalar(
                out=t[:, :], in0=ssum[:, :], scalar1=-2.0, scalar2=float(K),
                op0=_MULT, op1=_ADD,
            )
            qd = stats.tile([P, NB], _F32)
            nc.vector.tensor_tensor(out=qd[:, :], in0=t[:, :], in1=qsum[:, :], op=_ADD)
            md = stats.tile([P, NB], _F32)
            nc.vector.tensor_scalar(
                out=md[:, :], in0=mx[:, :], scalar1=-1.0, scalar2=1.0,
                op0=_MULT, op1=_ADD,
            )
            m2 = stats.tile([P, NB], _F32)
            nc.vector.tensor_tensor(out=m2[:, :], in0=md[:, :], in1=md[:, :], op=_MULT)
            num = stats.tile([P, NB], _F32)
            nc.vector.tensor_scalar(
                out=num[:, :], in0=m2[:, :], scalar1=2.0, scalar2=None, op0=_MULT,
            )
            num2 = stats.tile([P, NB], _F32)
            nc.vector.tensor_tensor(out=num2[:, :], in0=num[:, :], in1=qd[:, :], op=_SUB)
            rs = stats.tile([P, NB], _F32)
            nc.vector.reciprocal(rs[:, :], sd[:, :])
            frac = stats.tile([P, NB], _F32)
            nc.vector.tensor_tensor(out=frac[:, :], in0=num2[:, :], in1=rs[:, :], op=_MULT)
            base = stats.tile([P, NB], _F32)
            nc.vector.tensor_tensor(out=base[:, :], in0=sd[:, :], in1=md[:, :], op=_SUB)
            rloss = stats.tile([P, NB], _F32)
            nc.vector.tensor_tensor(out=rloss[:, :], in0=base[:, :], in1=frac[:, :], op=_ADD)
            rowsum = stats.tile([P, 1], _F32)
            nc.vector.tensor_reduce(out=rowsum[:, :], in_=rloss[:, :], axis=_AX, op=_ADD)
            # cross-partition sum via ones-matmul: [1,1] = rowsum^T @ ones
            psc = ppf.tile([1, 1], _F32)
            nc.tensor.matmul(
                psc[:, :], lhsT=rowsum[:, :], rhs=ones[:, :], start=True, stop=True
            )
            outsb = stats.tile([1, 1], _F32)
            nc.vector.tensor_copy(out=outsb[:, :], in_=psc[:, :])
            nc.sync.dma_start(out=out[:, :], in_=outsb[:, :])

    nc.finalize()  # Bacc: runs wait-splitting + register allocation passes
    return nc


def _prep_inputs(x, labels, centers):
    """Host-side sharding/layout prep. Returns (in_maps, wc)."""
    labels = np.asarray(labels).astype(np.int64)
    x = np.ascontiguousarray(np.asarray(x, dtype=np.float32))
    centers = np.asarray(centers, dtype=np.float32)

    perm = np.argsort(labels, kind="stable")
    ls = labels[perm]

    # per-core class windows
    starts, spans = [], []
    for i in range(NCORES):
        seg = ls[i * SHARD : (i + 1) * SHARD]
        lo, hi = int(seg[0]), int(seg[-1])
        starts.append(lo)
        spans.append(hi - lo + 1)
    w = max(max(spans), 16)  # >=16 classes so matmul N >= 256 (fp32r full rate)
    assert w * K <= 512, f"class span {w} too large for single PSUM bank"
    wc = w * K
    starts = [min(s, C - w) for s in starts]

    centersT = np.ascontiguousarray(centers.reshape(C * K, D).T)  # [D, C*K]

    in_maps = []
    for i in range(NCORES):
        rows = perm[i * SHARD : (i + 1) * SHARD]
        xsT = np.ascontiguousarray(x[rows].T)                     # [D, SHARD]
        # window, k-major columns: col j = k*w + c  ->  centersT col 16*(start+c)+k
        win = centersT[:, K * starts[i] : K * (starts[i] + w)]     # [D, w*K] c-major
        cwin = np.ascontiguousarray(
            win.reshape(D, w, K).transpose(0, 2, 1).reshape(D, wc)
        )
        lab_local = (ls[i * SHARD : (i + 1) * SHARD] - starts[i]).astype(np.float32)
        lab_dev = np.ascontiguousarray(lab_local.reshape(NB, P).T)  # [P, NB]
        in_maps.append({"xT": xsT, "cw": cwin, "lab": lab_dev})
    return in_maps, wc


def kernel(x, labels, centers):
    in_maps, wc = _prep_inputs(x, labels, centers)
    nc = _build_program(wc)
    res = run_bass_kernel_spmd(nc, in_maps, core_ids=list(range(NCORES)))
    total = sum(float(r["out"][0, 0]) for r in res.results)
    return np.float32(total / B)


# revision 9
# speedup vs baseline: 1.9124x; 1.3142x over previous
"""Trainium2 Bass kernel for CenterWoParamMultiCosineNearLoss.

loss = mean_b [ S_b - m_b + (2*m_b^2 - Q_b) / S_b ]   where, per sample b,
  d_k = 1 - <x_b, c_{label_b, k}>  (k = 0..15 sub-centers of own class)
  S = sum_k d_k, Q = sum_k d_k^2, m = min_k d_k
(algebraically identical to the reference's term1+term2; verified exactly).

Sharding: samples are sorted by label on the host (the loss is a mean over
samples, hence permutation-invariant) and split into 8 contiguous shards of
1024 — i.e. data-parallel with class-clustered assignment. Each core's shard
then spans only ~13 consecutive classes, so the device matmul per core is
x_shard^T [1024d x 1024b] against a small window of transposed centers
[1024d x 16*W cols] instead of all 90*16=1440 columns. Per-row selection of
the 16 own-class columns is done on device with an iota==label one-hot mask
and a reduction over the class axis (window columns are laid out k-major so
the reduce is contiguous). Per-row sum/sumsq go through the scalar engine's
accumulate path; each core emits its partial row-loss sum and the host
all-reduces the 8 scalars into the mean.
"""

import os
import sys

import numpy as np

for _p in ("/opt/trn_rl_repo", "/root/.axon_site/_ro/trn_rl_repo"):
    if os.path.isdir(_p) and _p not in sys.path:
        sys.path.append(_p)

import concourse.bass as bass  # noqa: E402
import concourse.tile as tile  # noqa: E402
from concourse import bacc  # noqa: E402
from concourse import mybir  # noqa: E402
from concourse.bass_utils import run_bass_kernel_spmd  # noqa: E402

P = 128          # SBUF partitions
B = 8192         # batch
D = 1024         # feature dim
C = 90           # classes
K = 16           # sub-centers per class
NCORES = 8
SHARD = B // NCORES          # 1024 samples per core
NB = SHARD // P              # 8 row-blocks per core
KT = D // P                  # 8 contraction tiles

_F32 = mybir.dt.float32
_F16 = mybir.dt.float16

_ADD = mybir.AluOpType.add
_MULT = mybir.AluOpType.mult
_SUB = mybir.AluOpType.subtract
_MAX = mybir.AluOpType.max
_EQ = mybir.AluOpType.is_equal
_AX = mybir.AxisListType.X


def _build_program(wc: int):
    """One SPMD program for all 8 cores. wc = window width in columns (K*w)."""
    w = wc // K
    nc = bacc.Bacc(None, target_bir_lowering=False)
    xT = nc.declare_dram_parameter("xT", [D, SHARD], _F16, isOutput=False)
    cw = nc.declare_dram_parameter("cw", [D, wc], _F16, isOutput=False)
    lab = nc.declare_dram_parameter("lab", [P, NB], _F32, isOutput=False)
    out = nc.declare_dram_parameter("out", [1, 1], _F32, isOutput=True)

    with tile.TileContext(nc) as tc:
        with (
            tc.tile_pool(name="const", bufs=1) as const,
            tc.tile_pool(name="cwp", bufs=1) as cwp,
            tc.tile_pool(name="xp", bufs=4) as xp,
            tc.tile_pool(name="maskp", bufs=NB) as maskp,
            tc.tile_pool(name="work", bufs=4) as work,
            tc.tile_pool(name="stats", bufs=1) as stats,
            tc.tile_pool(name="pp", bufs=6, space="PSUM") as pp,
            tc.tile_pool(name="ppf", bufs=1, space="PSUM") as ppf,
        ):
            # constants.  window columns are k-major: col j = k*w + c, so the
            # class id at column j is (j mod w)
            colc = const.tile([P, wc], _F32)
            nc.gpsimd.iota(
                colc[:, :], pattern=[[0, K], [1, w]], channel_multiplier=0,
                allow_small_or_imprecise_dtypes=True,
            )
            ones = const.tile([P, 1], _F32)
            nc.vector.memset(ones[:, :], 1.0)
            # small DMA via SWDGE (single queue/sem) — an HWDGE transfer fans
            # out across queues and overflows the consumer's wait slots
            labt = const.tile([P, NB], _F32)
            nc.gpsimd.dma_start(out=labt[:, :], in_=lab[:, :])

            # all 8 k-tiles of the centers window: [d_local, ktile, col]
            cwt = cwp.tile([P, KT, wc], _F16)
            nc.sync.dma_start(
                out=cwt[:, :, :], in_=cw[:, :].rearrange("(k p) w -> p k w", p=P)
            )

            # one-hot masks for every block up-front (only depend on labt)
            masks = []
            for i in range(NB):
                mask = maskp.tile([P, wc], _F32)
                nc.vector.tensor_scalar(
                    out=mask[:, :], in0=colc[:, :], scalar1=labt[:, i : i + 1],
                    scalar2=None, op0=_EQ,
                )
                masks.append(mask)

            ssum = stats.tile([P, NB], _F32)   # per-row sum of selected cos
            qsum = stats.tile([P, NB], _F32)   # per-row sum of selected cos^2
            mx = stats.tile([P, NB], _F32)     # per-row max of selected cos

            for j in range(NB // 2):
                # two row-blocks per DMA: 1KB contiguous runs instead of 512B
                xbt = xp.tile([P, KT, 2 * P], _F16)
                nc.sync.dma_start(
                    out=xbt[:, :, :],
                    in_=xT[:, j * 2 * P : (j + 1) * 2 * P].rearrange(
                        "(k p) b -> p k b", p=P
                    ),
                )
                for h in range(2):
                    i = 2 * j + h
                    ps = pp.tile([P, wc], _F32)
                    for k in range(KT):
                        nc.tensor.matmul(
                            ps[:, :],
                            lhsT=xbt[:, k, h * P : (h + 1) * P],
                            rhs=cwt[:, k, :],
                            start=(k == 0),
                            stop=(k == KT - 1),
                        )
                    sm = work.tile([P, wc], _F32)
                    nc.vector.tensor_tensor(
                        out=sm[:, :], in0=ps[:, :], in1=masks[i][:, :], op=_MULT
                    )
                    # collapse the class axis (contiguous, k-major layout)
                    dsel = work.tile([P, K], _F32)
                    nc.vector.tensor_reduce(
                        out=dsel[:, :],
                        in_=sm[:, :].rearrange("p (k c) -> p k c", c=w),
                        axis=_AX, op=_ADD,
                    )
                    # sum and sum-of-squares on the (idle) scalar engine via
                    # its accumulate path; max stays on vector
                    sq = work.tile([P, K], _F32)
                    nc.scalar.activation(
                        out=sq[:, :], in_=dsel[:, :],
                        func=mybir.ActivationFunctionType.Square,
                        accum_out=qsum[:, i : i + 1],
                    )
                    cp = work.tile([P, K], _F32)
                    nc.scalar.activation(
                        out=cp[:, :], in_=dsel[:, :],
                        func=mybir.ActivationFunctionType.Copy,
                        accum_out=ssum[:, i : i + 1],
                    )
                    nc.vector.tensor_reduce(
                        out=mx[:, i : i + 1], in_=dsel[:, :], axis=_AX, op=_MAX,
                    )

            # epilogue on [P, NB]: d = 1 - s  =>
            #   S = K - ssum; Q = K - 2*ssum + qsum; m = 1 - mx
            #   rowloss = S - m + (2*m^2 - Q) / S
            sd = stats.tile([P, NB], _F32)
            nc.vector.tensor_scalar(
                out=sd[:, :], in0=ssum[:, :], scalar1=-1.0, scalar2=float(K),
                op0=_MULT, op1=_ADD,
            )
            t = stats.tile([P, NB], _F32)
            nc.vector.tensor_scalar(
                out=t[:, :], in0=ssum[:, :], scalar1=-2.0, scalar2=float(K),
                op0=_MULT, op1=_ADD,
            )
            qd = stats.tile([P, NB], _F32)
            nc.vector.tensor_tensor(out=qd[:, :], in0=t[:, :], in1=qsum[:, :], op=_ADD)
            md = stats.tile([P, NB], _F32)
            nc.vector.tensor_scalar(
                out=md[:, :], in0=mx[:, :], scalar1=-1.0, scalar2=1.0,
                op0=_MULT, op1=_ADD,
            )
            m2 = stats.tile([P, NB], _F32)
            nc.vector.tensor_tensor(out=m2[:, :], in0=md[:, :], in1=md[:, :], op=_MULT)
            num = stats.tile([P, NB], _F32)
            nc.vector.tensor_scalar(
                out=num[:, :], in0=m2[:, :], scalar1=2.0, scalar2=None, op0=_MULT,
            )
            num2 = stats.tile([P, NB], _F32)
            nc.vector.tensor_tensor(out=num2[:, :], in0=num[:, :], in1=qd[:, :], op=_SUB)
            rs = stats.tile([P, NB], _F32)
            nc.vector.reciprocal(rs[:, :], sd[:, :])
            frac = stats.tile([P, NB], _F32)
            nc.vector.tensor_tensor(out=frac[:, :], in0=num2[:, :], in1=rs[:, :], op=_MULT)
            base = stats.tile([P, NB], _F32)
            nc.vector.tensor_tensor(out=base[:, :], in0=sd[:, :], in1=md[:, :], op=_SUB)
            rloss = stats.tile([P, NB], _F32)
            nc.vector.tensor_tensor(out=rloss[:, :], in0=base[:, :], in1=frac[:, :], op=_ADD)
            rowsum = stats.tile([P, 1], _F32)
            nc.vector.tensor_reduce(out=rowsum[:, :], in_=rloss[:, :], axis=_AX, op=_ADD)
            # cross-partition sum via ones-matmul: [1,1] = rowsum^T @ ones
            psc = ppf.tile([1, 1], _F32)
            nc.tensor.matmul(
                psc[:, :], lhsT=rowsum[:, :], rhs=ones[:, :], start=True, stop=True
            )
            outsb = stats.tile([1, 1], _F32)
            nc.vector.tensor_copy(out=outsb[:, :], in_=psc[:, :])
            nc.sync.dma_start(out=out[:, :], in_=outsb[:, :])

    nc.finalize()  # Bacc: runs wait-splitting + register allocation passes
    return nc


def _prep_inputs(x, labels, centers):
    """Host-side sharding/layout prep. Returns (in_maps, wc)."""
    labels = np.asarray(labels).astype(np.int64)
    x = np.ascontiguousarray(np.asarray(x, dtype=np.float32))
    centers = np.asarray(centers, dtype=np.float32)

    perm = np.argsort(labels, kind="stable")
    ls = labels[perm]

    # per-core class windows
    starts, spans = [], []
    for i in range(NCORES):
        seg = ls[i * SHARD : (i + 1) * SHARD]
        lo, hi = int(seg[0]), int(seg[-1])
        starts.append(lo)
        spans.append(hi - lo + 1)
    w = max(spans)
    assert w * K <= 512, f"class span {w} too large for single PSUM bank"
    wc = w * K
    starts = [min(s, C - w) for s in starts]

    centersT = np.ascontiguousarray(centers.reshape(C * K, D).T)  # [D, C*K]

    in_maps = []
    for i in range(NCORES):
        rows = perm[i * SHARD : (i + 1) * SHARD]
        xsT = np.ascontiguousarray(x[rows].T.astype(np.float16))  # [D, SHARD]
        # window, k-major columns: col j = k*w + c  ->  centersT col 16*(start+c)+k
        win = centersT[:, K * starts[i] : K * (starts[i] + w)]     # [D, w*K] c-major
        cwin = np.ascontiguousarray(
            win.reshape(D, w, K).transpose(0, 2, 1).reshape(D, wc).astype(np.float16)
        )
        lab_local = (ls[i * SHARD : (i + 1) * SHARD] - starts[i]).astype(np.float32)
        lab_dev = np.ascontiguousarray(lab_local.reshape(NB, P).T)  # [P, NB]
        in_maps.append({"xT": xsT, "cw": cwin, "lab": lab_dev})
    return in_maps, wc


def kernel(x, labels, centers):
    in_maps, wc = _prep_inputs(x, labels, centers)
    nc = _build_program(wc)
    res = run_bass_kernel_spmd(nc, in_maps, core_ids=list(range(NCORES)))
    total = sum(float(r["out"][0, 0]) for r in res.results)
    return np.float32(total / B)
